# revision 27
# baseline (speedup 1.0000x reference)
"""CapsuleLayer dynamic-routing kernel for 8 TRN2 NeuronCores.

Strategy: shard R(=8192) across the 8 cores (R_local=1024). Per-core x/W
shards are then small enough (~7MB bf16) to be SBUF-resident, so the 671MB
u_hat intermediate never touches HBM. The routing recurrence is restructured
using linearity of a_ij in v_j:  b_ij(t) = u_hat . (v_0+...+v_{t-1}),
so each iteration is one fused pass over r-tiles:
  u_tile (PE matmuls, K=i=8) -> b=u.V (DVE) -> softmax (ACT exp + DVE)
  -> s_partial accumulation (DVE).
Cross-core: s_j partials ([128,160] fp32, 80KB) AllReduced after passes 0/1;
pass-2 partials are summed + squashed on host.

Host-side prep swizzles x/W into the 32-aligned padded layout required for
K=8 PE matmuls (partition = (r%4)*32 + i) and converts to bf16.
"""
import numpy as np
import ml_dtypes
from contextlib import ExitStack

import concourse.bass as bass
import concourse.bacc as bacc
import concourse.tile as tile
from concourse import mybir
from concourse.bass_utils import run_bass_kernel_spmd

B, C, R, I, O = 128, 10, 8192, 8, 16
NCORES = 8
RL = R // NCORES          # 1024 r's per core
RQ = RL // 4              # 256
RT = 8                    # r's per iteration tile
NTILES = RL // RT         # 128
F32 = mybir.dt.float32
BF16 = mybir.dt.bfloat16
AX = mybir.AxisListType.X


def _bc(ap, dims):
    """Build a broadcast/permuted view of an AP. dims: list of entries that are
    either an int index into ap.ap (reuse that dim) or a tuple (0, count) for a
    broadcast dim."""
    new = []
    for d in dims:
        if isinstance(d, tuple):
            new.append([d[0], d[1]])
        else:
            new.append(ap.ap[d])
    return bass.AP(tensor=ap.tensor, offset=ap.offset, ap=new)


def _squash_emit(nc, pool, s_ap, v_out, bias_eps):
    """v_out[:, c, o] = squash(s_ap[:, c, o]) over o. All [128, C, O] f32."""
    m2 = pool.tile([B, C, O], F32, tag="sq_m2")
    nc.vector.tensor_mul(m2[:], s_ap, s_ap)
    sq = pool.tile([B, C], F32, tag="sq_sq")
    nc.vector.reduce_sum(out=sq[:], in_=m2[:], axis=AX)
    rt_ = pool.tile([B, C], F32, tag="sq_rt")
    nc.scalar.activation(rt_[:], sq[:], mybir.ActivationFunctionType.Sqrt,
                         bias=bias_eps[:], scale=1.0)
    d1 = pool.tile([B, C], F32, tag="sq_d1")
    nc.vector.tensor_scalar_add(d1[:], sq[:], 1.0)
    den = pool.tile([B, C], F32, tag="sq_den")
    nc.vector.tensor_mul(den[:], d1[:], rt_[:])
    rec = pool.tile([B, C], F32, tag="sq_rec")
    nc.vector.reciprocal(rec[:], den[:])
    scale = pool.tile([B, C], F32, tag="sq_scale")
    nc.vector.tensor_mul(scale[:], sq[:], rec[:])
    # v = s * scale  (scale broadcast over o)
    nc.vector.tensor_mul(v_out, s_ap, _bc(scale, [0, 1, (0, O)]))


def build_nc():
    nc = bacc.Bacc(None, num_devices=NCORES)
    # Dense (unpadded) DRAM inputs: [ (m,i)=32, ... ]. They are scattered into
    # the 32-aligned SBUF layout by DMA placement; SBUF rows 8..31 of each
    # 32-row group are never read (matmuls use K=8 slices).
    xq_d = nc.declare_dram_parameter("xq", [32, RQ * B], BF16, isOutput=False)
    # W ships in its NATURAL per-core layout [C, RL, I, O] (host only casts to
    # bf16); the load DMA scatters it into the [m*32+i, rq, (c,o)] SBUF layout
    # (r is grouped as m = r // RQ, rq = r % RQ — the grouping is arbitrary as
    # long as x uses the same one).
    wn_d = nc.declare_dram_parameter("wn", [C, RL * I * O], BF16, isOutput=False)
    out_d = nc.declare_dram_parameter("s2", [B, C * O], F32, isOutput=True)

    with ExitStack() as ctx:
        tc = ctx.enter_context(tile.TileContext(nc))
        consts = ctx.enter_context(tc.tile_pool(name="consts", bufs=1))
        psum = ctx.enter_context(tc.tile_pool(name="psum", bufs=2, space="PSUM"))
        work = ctx.enter_context(tc.tile_pool(name="work", bufs=6))
        sq = ctx.enter_context(tc.tile_pool(name="sq", bufs=1))
        acc = ctx.enter_context(tc.tile_pool(name="acc", bufs=1))
        dram = ctx.enter_context(tc.tile_pool(name="dram", bufs=1, space="DRAM"))

        xq = consts.tile([128, RQ, B], BF16)
        wq = consts.tile([128, C, RQ, O], BF16)
        wn_v = wn_d[:].rearrange("c (r i o) -> c r i o", i=I, o=O)
        for m in range(4):
            nc.sync.dma_start(
                out=xq[:][m * 32:m * 32 + I],
                in_=xq_d[:].rearrange("p (q b) -> p q b", b=B)[m * I:(m + 1) * I])
            # scatter W: per (m, c) a 3-dim DMA [i, rq, o] (o-runs contiguous
            # in the natural layout).
            for c in range(C):
                nc.sync.dma_start(
                    out=wq[:][m * 32:m * 32 + I, c],
                    in_=_bc(wn_v[c], [1, 0, 2])[:, m * RQ:(m + 1) * RQ])

        bias_eps = acc.tile([B, 1], F32)
        nc.vector.memset(bias_eps[:], 1e-8)
        bias_zero = acc.tile([B, 1], F32)
        nc.vector.memset(bias_zero[:], 0.0)

        V = acc.tile([B, C, O], F32)      # running sum of v_t
        sfull = acc.tile([B, C, O], F32)  # AllReduced s_j

        # ---------------- pass 0: c uniform -> s0 = 0.1 * sum_r u_r ----------
        # One psum accumulator per m-residue (mms in an accumulation group
        # must share a tile_position / psum bank).
        s_acc0 = acc.tile([B, C, O], F32)
        for m in range(4):
            s0t = psum.tile([B, RT, C * O], F32, tag="u_ps")
            s0m = s0t[:, 0]
            for rq in range(RQ):
                nc.tensor.matmul(
                    s0m, xq[m * 32:m * 32 + 8, rq], wq[:][m * 32:m * 32 + 8, :, rq],
                    start=(rq == 0), stop=(rq == RQ - 1),
                    tile_position=(m * 32, 0))
            if m == 0:
                nc.vector.tensor_scalar_mul(
                    s_acc0[:].rearrange("b c o -> b (c o)"), s0m, 1.0 / C)
            else:
                nc.vector.scalar_tensor_tensor(
                    out=s_acc0[:].rearrange("b c o -> b (c o)"), in0=s0m,
                    scalar=1.0 / C, in1=s_acc0[:].rearrange("b c o -> b (c o)"),
                    op0=mybir.AluOpType.mult, op1=mybir.AluOpType.add)

        # helper: AllReduce src (a [B, C, O]-ordered AP, maybe strided) -> sfull
        def allreduce(idx, src_ap, stage=None):
            ar_in = dram.tile([B, C * O], F32, tag=f"ar_in{idx}")
            ar_out = dram.tile([B, C * O], F32, tag=f"ar_out{idx}",
                               addr_space="Shared")
            if stage is not None:
                nc.vector.tensor_copy(
                    stage[:].rearrange("b (c o) -> b c o", o=O), src_ap)
                src_ap = stage[:]
            nc.gpsimd.dma_start(
                out=ar_in[:].rearrange("b (c o) -> b c o", o=O)
                if len(src_ap.shape) == 3 else ar_in[:],
                in_=src_ap)
            nc.gpsimd.collective_compute(
                "AllReduce", mybir.AluOpType.add,
                replica_groups=[list(range(NCORES))],
                ins=[ar_in[:].opt()], outs=[ar_out[:].opt()])
            nc.gpsimd.dma_start(out=sfull[:].rearrange("b c o -> b (c o)"),
                                in_=ar_out[:])

        allreduce(0, s_acc0[:])
        _squash_emit(nc, sq, sfull[:], V[:], bias_eps)  # V = v0

        # ---------------- routing passes 1 and 2 -----------------------------
        # Tile math in [b, o, r, c] free order: both the o-contraction (logits)
        # and the r-contraction (s accumulation) become contiguous halving
        # tree-adds (2x DVE mode) instead of 1x tensor_reduce. The t2 multiply
        # runs on the otherwise-idle Pool engine.
        for it in (1, 2):
            s_acc = acc.tile([B, O, C], F32, tag=f"s_acc{it}")
            V_exp = acc.tile([B, O, RT, C], BF16, tag=f"V_exp{it}")
            nc.vector.tensor_copy(
                V_exp[:], _bc(V[:], [0, 2, (0, RT), 1]))
            # 3-stage software pipeline: front-end (matmuls+u_sb+logit tree) of
            # tile i, softmax+t2 of tile i-1, s-accumulation of tile i-2 —
            # emitted interleaved so no engine head-waits on a fresh
            # cross-engine dependency.
            def stage_f(ti):
                m, q = ti // 32, ti % 32
                u_ps = psum.tile([B, RT, C * O], F32, tag="u_ps")
                for j in range(RT):
                    rq = RT * q + j       # this tile covers r = 4*rq + m
                    nc.tensor.matmul(
                        u_ps[:, j], xq[m * 32:m * 32 + 8, rq],
                        wq[:][m * 32:m * 32 + 8, :, rq], start=True, stop=True,
                        tile_position=(m * 32, 0))
                u_v = u_ps[:].rearrange("b r (c o) -> b r c o", o=O)
                u_sb = work.tile([B, O, RT, C], BF16, tag="u_sb")
                nc.scalar.copy(u_sb[:].rearrange("b o r c -> b r c o"), u_v)
                # logits: b[b,r,c] = sum_o u*V  (in-place tree over o)
                t = work.tile([B, O, RT, C], BF16, tag="t")
                nc.vector.tensor_mul(t[:], u_sb[:], V_exp[:])
                nc.vector.tensor_add(t[:, :8], t[:, :8], t[:, 8:])
                nc.vector.tensor_add(t[:, :4], t[:, :4], t[:, 4:8])
                nc.vector.tensor_add(t[:, :2], t[:, :2], t[:, 2:4])
                nc.vector.tensor_add(t[:, 0], t[:, 0], t[:, 1])
                return u_sb, t

            def stage_m(st):
                u_sb, t = st
                # softmax over c (no max-subtraction; |b| is small)
                e = work.tile([B, RT, C], BF16, tag="e")
                nc.scalar.activation(e[:], t[:, 0],
                                     mybir.ActivationFunctionType.Exp,
                                     bias=bias_zero[:], scale=1.0)
                ssum = work.tile([B, RT], F32, tag="ssum")
                nc.vector.reduce_sum(out=ssum[:], in_=e[:], axis=AX)
                nrec = work.tile([B, RT], F32, tag="nrec")
                nc.vector.reciprocal(nrec[:], ssum[:])
                w = work.tile([B, RT, C], BF16, tag="w")
                nc.vector.tensor_mul(w[:], e[:], _bc(nrec, [0, 1, (0, C)]))
                # s-side multiply on the Pool engine (w broadcast over o via AP)
                t2 = work.tile([B, O, RT, C], BF16, tag="t2")
                nc.gpsimd.tensor_mul(t2[:], u_sb[:], _bc(w, [0, (0, O), 1, 2]))
                return t2

            def stage_b(ti, t2):
                # s += sum_r w*u  (in-place tree over r)
                nc.vector.tensor_add(t2[:, :, :4], t2[:, :, :4], t2[:, :, 4:])
                nc.vector.tensor_add(t2[:, :, :2], t2[:, :, :2], t2[:, :, 2:4])
                nc.vector.tensor_add(t2[:, :, 0], t2[:, :, 0], t2[:, :, 1])
                if ti == 0:
                    nc.vector.tensor_copy(s_acc[:], t2[:, :, 0])
                else:
                    nc.vector.tensor_add(s_acc[:], s_acc[:], t2[:, :, 0])

            f_prev = m_prev = None
            for ti in range(NTILES + 2):
                f_cur = stage_f(ti) if ti < NTILES else None
                m_cur = stage_m(f_prev) if f_prev is not None else None
                if m_prev is not None:
                    stage_b(ti - 2, m_prev)
                f_prev, m_prev = f_cur, m_cur
            s_stage = acc.tile([B, C * O], F32, tag=f"s_stage{it}")
            if it == 1:
                allreduce(1, s_acc[:].rearrange("b o c -> b c o"),
                          stage=s_stage)
                v1 = sq.tile([B, C, O], F32, tag="v1")
                _squash_emit(nc, sq, sfull[:], v1[:], bias_eps)
                nc.vector.tensor_add(V[:], V[:], v1[:])
            else:
                nc.vector.tensor_copy(
                    s_stage[:].rearrange("b (c o) -> b c o", o=O),
                    s_acc[:].rearrange("b o c -> b c o"))
                nc.gpsimd.dma_start(out=out_d[:], in_=s_stage[:])
    nc.compile()
    return nc


_PREP_CACHE = {}


def _prep_shards(x, w):
    """x -> [core, 4(m), I, RQ, B] bf16 (r block-grouped: m = r//RQ);
    w -> per-core natural [C, RL, I, O] bf16. Runs on jax-CPU (multithreaded).
    """
    import jax
    import jax.numpy as jnp
    if "fn" not in _PREP_CACHE:
        cpu = jax.devices("cpu")[0]

        def _prep(x, w):
            xr = x.reshape(B, NCORES, 4, RQ, I).transpose(1, 2, 4, 3, 0)
            return xr.astype(jnp.bfloat16), w.astype(jnp.bfloat16)

        _PREP_CACHE["fn"] = jax.jit(_prep, device=cpu)
    xq, w16 = _PREP_CACHE["fn"](x, w)
    xq = np.asarray(xq)
    w16 = np.asarray(w16)
    maps = []
    for core in range(NCORES):
        r0 = core * RL
        maps.append({"xq": xq[core].reshape(32, RQ * B),
                     "wn": np.ascontiguousarray(
                         w16[:, r0:r0 + RL]).reshape(C, RL * I * O)})
    return maps


_NC_CACHE = {}


def _postprocess(results):
    """results: list of per-core output dicts -> full [B, C, O] output."""
    s2 = np.zeros((B, C * O), dtype=np.float32)
    for i in range(NCORES):
        s2 += np.asarray(results[i]["s2"], dtype=np.float32)
    s2 = s2.reshape(B, C, O)
    sq = np.sum(s2 * s2, axis=-1, keepdims=True)
    v = (sq / (1.0 + sq)) * s2 / np.sqrt(sq + 1e-8)
    return v.astype(np.float32)


def kernel(x, route_weights, _trace=False):
    x = np.asarray(x, dtype=np.float32)
    w = np.asarray(route_weights, dtype=np.float32)
    in_maps = _prep_shards(x, w)
    if "nc" not in _NC_CACHE:
        _NC_CACHE["nc"] = build_nc()
    nc = _NC_CACHE["nc"]
    kw = {}
    if _trace:
        kw = dict(trace=True)
    try:
        res = run_bass_kernel_spmd(nc, in_maps, core_ids=list(range(NCORES)), **kw)
    except ModuleNotFoundError:
        res = run_bass_kernel_spmd(nc, in_maps, core_ids=list(range(NCORES)))
    if _trace:
        kernel._last_exec_ns = getattr(res, "exec_time_ns", None)
    return _postprocess(res.results)



# revision 31
# speedup vs baseline: 1.1538x; 1.1538x over previous
"""CapsuleLayer dynamic-routing kernel for 8 TRN2 NeuronCores.

Strategy: shard R(=8192) across the 8 cores (R_local=1024). Per-core x/W
shards are then small enough (~7MB bf16) to be SBUF-resident, so the 671MB
u_hat intermediate never touches HBM. The routing recurrence is restructured
using linearity of a_ij in v_j:  b_ij(t) = u_hat . (v_0+...+v_{t-1}),
so each iteration is one fused pass over r-tiles:
  u_tile (PE matmuls, K=i=8) -> b=u.V (DVE) -> softmax (ACT exp + DVE)
  -> s_partial accumulation (DVE).
Cross-core: s_j partials ([128,160] fp32, 80KB) AllReduced after passes 0/1;
pass-2 partials are summed + squashed on host.

Host-side prep swizzles x/W into the 32-aligned padded layout required for
K=8 PE matmuls (partition = (r%4)*32 + i) and converts to bf16.
"""
import numpy as np
import ml_dtypes
from contextlib import ExitStack

import concourse.bass as bass
import concourse.bacc as bacc
import concourse.tile as tile
from concourse import mybir
from concourse.bass_utils import run_bass_kernel_spmd

B, C, R, I, O = 128, 10, 8192, 8, 16
NCORES = 8
RL = R // NCORES          # 1024 r's per core
RQ = RL // 4              # 256
RT = 8                    # r's per iteration tile
NTILES = RL // RT         # 128
F32 = mybir.dt.float32
BF16 = mybir.dt.bfloat16
AX = mybir.AxisListType.X


def _bc(ap, dims):
    """Build a broadcast/permuted view of an AP. dims: list of entries that are
    either an int index into ap.ap (reuse that dim) or a tuple (0, count) for a
    broadcast dim."""
    new = []
    for d in dims:
        if isinstance(d, tuple):
            new.append([d[0], d[1]])
        else:
            new.append(ap.ap[d])
    return bass.AP(tensor=ap.tensor, offset=ap.offset, ap=new)


def _squash_emit(nc, pool, s_ap, v_out, bias_eps):
    """v_out[:, c, o] = squash(s_ap[:, c, o]) over o. All [128, C, O] f32."""
    m2 = pool.tile([B, C, O], F32, tag="sq_m2")
    nc.vector.tensor_mul(m2[:], s_ap, s_ap)
    sq = pool.tile([B, C], F32, tag="sq_sq")
    nc.vector.reduce_sum(out=sq[:], in_=m2[:], axis=AX)
    rt_ = pool.tile([B, C], F32, tag="sq_rt")
    nc.scalar.activation(rt_[:], sq[:], mybir.ActivationFunctionType.Sqrt,
                         bias=bias_eps[:], scale=1.0)
    d1 = pool.tile([B, C], F32, tag="sq_d1")
    nc.vector.tensor_scalar_add(d1[:], sq[:], 1.0)
    den = pool.tile([B, C], F32, tag="sq_den")
    nc.vector.tensor_mul(den[:], d1[:], rt_[:])
    rec = pool.tile([B, C], F32, tag="sq_rec")
    nc.vector.reciprocal(rec[:], den[:])
    scale = pool.tile([B, C], F32, tag="sq_scale")
    nc.vector.tensor_mul(scale[:], sq[:], rec[:])
    # v = s * scale  (scale broadcast over o)
    nc.vector.tensor_mul(v_out, s_ap, _bc(scale, [0, 1, (0, O)]))


def build_nc():
    nc = bacc.Bacc(None, num_devices=NCORES)
    # Dense (unpadded) DRAM inputs: [ (m,i)=32, ... ]. They are scattered into
    # the 32-aligned SBUF layout by DMA placement; SBUF rows 8..31 of each
    # 32-row group are never read (matmuls use K=8 slices).
    xq_d = nc.declare_dram_parameter("xq", [32, RQ * B], BF16, isOutput=False)
    # W ships in its NATURAL per-core layout [C, RL, I, O] (host only casts to
    # bf16); the load DMA scatters it into the [m*32+i, rq, (c,o)] SBUF layout
    # (r is grouped as m = r // RQ, rq = r % RQ — the grouping is arbitrary as
    # long as x uses the same one).
    wn_d = nc.declare_dram_parameter("wn", [C, RL * I * O], BF16, isOutput=False)
    out_d = nc.declare_dram_parameter("s2", [B, C * O], F32, isOutput=True)

    with ExitStack() as ctx:
        tc = ctx.enter_context(tile.TileContext(nc))
        consts = ctx.enter_context(tc.tile_pool(name="consts", bufs=1))
        psum = ctx.enter_context(tc.tile_pool(name="psum", bufs=2, space="PSUM"))
        work = ctx.enter_context(tc.tile_pool(name="work", bufs=3))
        sq = ctx.enter_context(tc.tile_pool(name="sq", bufs=1))
        acc = ctx.enter_context(tc.tile_pool(name="acc", bufs=1))
        dram = ctx.enter_context(tc.tile_pool(name="dram", bufs=1, space="DRAM"))

        xq = consts.tile([128, RQ, B], BF16)
        wq = consts.tile([128, C, RQ, O], BF16)
        wn_v = wn_d[:].rearrange("c (r i o) -> c r i o", i=I, o=O)
        for m in range(4):
            nc.sync.dma_start(
                out=xq[:][m * 32:m * 32 + I],
                in_=xq_d[:].rearrange("p (q b) -> p q b", b=B)[m * I:(m + 1) * I])
            # scatter W: per (m, c) a 3-dim DMA [i, rq, o] (o-runs contiguous
            # in the natural layout).
            for c in range(C):
                nc.sync.dma_start(
                    out=wq[:][m * 32:m * 32 + I, c],
                    in_=_bc(wn_v[c], [1, 0, 2])[:, m * RQ:(m + 1) * RQ])

        bias_eps = acc.tile([B, 1], F32)
        nc.vector.memset(bias_eps[:], 1e-8)
        bias_zero = acc.tile([B, 1], F32)
        nc.vector.memset(bias_zero[:], 0.0)

        V = acc.tile([B, C, O], F32)      # running sum of v_t
        sfull = acc.tile([B, C, O], F32)  # AllReduced s_j

        # ---------------- pass 0: c uniform -> s0 = 0.1 * sum_r u_r ----------
        # One psum accumulator per m-residue (mms in an accumulation group
        # must share a tile_position / psum bank).
        s_acc0 = acc.tile([B, C, O], F32)
        for m in range(4):
            s0t = psum.tile([B, RT, C * O], F32, tag="u_ps")
            s0m = s0t[:, 0]
            for rq in range(RQ):
                nc.tensor.matmul(
                    s0m, xq[m * 32:m * 32 + 8, rq], wq[:][m * 32:m * 32 + 8, :, rq],
                    start=(rq == 0), stop=(rq == RQ - 1),
                    tile_position=(m * 32, 0))
            if m == 0:
                nc.vector.tensor_scalar_mul(
                    s_acc0[:].rearrange("b c o -> b (c o)"), s0m, 1.0 / C)
            else:
                nc.vector.scalar_tensor_tensor(
                    out=s_acc0[:].rearrange("b c o -> b (c o)"), in0=s0m,
                    scalar=1.0 / C, in1=s_acc0[:].rearrange("b c o -> b (c o)"),
                    op0=mybir.AluOpType.mult, op1=mybir.AluOpType.add)

        # helper: AllReduce src (a [B, C, O]-ordered AP, maybe strided) -> sfull
        def allreduce(idx, src_ap, stage=None):
            ar_in = dram.tile([B, C * O], F32, tag=f"ar_in{idx}")
            ar_out = dram.tile([B, C * O], F32, tag=f"ar_out{idx}",
                               addr_space="Shared")
            if stage is not None:
                nc.vector.tensor_copy(
                    stage[:].rearrange("b (c o) -> b c o", o=O), src_ap)
                src_ap = stage[:]
            nc.gpsimd.dma_start(
                out=ar_in[:].rearrange("b (c o) -> b c o", o=O)
                if len(src_ap.shape) == 3 else ar_in[:],
                in_=src_ap)
            nc.gpsimd.collective_compute(
                "AllReduce", mybir.AluOpType.add,
                replica_groups=[list(range(NCORES))],
                ins=[ar_in[:].opt()], outs=[ar_out[:].opt()])
            nc.gpsimd.dma_start(out=sfull[:].rearrange("b c o -> b (c o)"),
                                in_=ar_out[:])

        allreduce(0, s_acc0[:])
        _squash_emit(nc, sq, sfull[:], V[:], bias_eps)  # V = v0

        # ---------------- routing passes 1 and 2 -----------------------------
        # Tile math in [b, o, r, c] free order: both the o-contraction (logits)
        # and the r-contraction (s accumulation) become contiguous halving
        # tree-adds (2x DVE mode) instead of 1x tensor_reduce. The t2 multiply
        # runs on the otherwise-idle Pool engine.
        RT2 = 2 * RT              # 16 r's per compute tile (2 psum tiles)
        NCT = NTILES // 2         # 64 compute tiles per pass
        for it in (1, 2):
            s_acc = acc.tile([B, O, C], F32, tag="s_acc")
            V_exp = acc.tile([B, O, RT2, C], BF16, tag="V_exp")
            nc.vector.tensor_copy(
                V_exp[:], _bc(V[:], [0, 2, (0, RT2), 1]))
            # 3-stage software pipeline: front-end (matmuls+u_sb+logit tree) of
            # compute tile i, softmax+t2 of i-1, s-accumulation of i-2 —
            # emitted interleaved so no engine head-waits on a fresh
            # cross-engine dependency.
            def stage_f(ci):
                u_sb = work.tile([B, O, RT2, C], BF16, tag="u_sb")
                for half in range(2):
                    ti = 2 * ci + half
                    m, q = ti // 32, ti % 32
                    u_ps = psum.tile([B, RT, C * O], F32, tag="u_ps")
                    for j in range(RT):
                        rq = RT * q + j   # this tile covers r = m*RQ + rq
                        nc.tensor.matmul(
                            u_ps[:, j], xq[m * 32:m * 32 + 8, rq],
                            wq[:][m * 32:m * 32 + 8, :, rq],
                            start=True, stop=True, tile_position=(m * 32, 0))
                    u_v = u_ps[:].rearrange("b r (c o) -> b r c o", o=O)
                    nc.scalar.copy(
                        u_sb[:, :, RT * half:RT * (half + 1)]
                        .rearrange("b o r c -> b r c o"), u_v)
                # logits: b[b,r,c] = sum_o u*V  (in-place tree over o)
                t = work.tile([B, O, RT2, C], BF16, tag="t")
                nc.vector.tensor_mul(t[:], u_sb[:], V_exp[:])
                nc.vector.tensor_add(t[:, :8], t[:, :8], t[:, 8:])
                nc.vector.tensor_add(t[:, :4], t[:, :4], t[:, 4:8])
                nc.vector.tensor_add(t[:, :2], t[:, :2], t[:, 2:4])
                nc.vector.tensor_add(t[:, 0], t[:, 0], t[:, 1])
                return u_sb, t

            def stage_m(st):
                u_sb, t = st
                # softmax over c (no max-subtraction; |b| is small)
                e = work.tile([B, RT2, C], BF16, tag="e")
                nc.scalar.activation(e[:], t[:, 0],
                                     mybir.ActivationFunctionType.Exp,
                                     bias=bias_zero[:], scale=1.0)
                ssum = work.tile([B, RT2], F32, tag="ssum")
                nc.vector.reduce_sum(out=ssum[:], in_=e[:], axis=AX)
                nrec = work.tile([B, RT2], F32, tag="nrec")
                nc.vector.reciprocal(nrec[:], ssum[:])
                nc.vector.tensor_mul(e[:], e[:], _bc(nrec, [0, 1, (0, C)]))
                # s-side multiply on the Pool engine (w broadcast over o via AP)
                t2 = work.tile([B, O, RT2, C], BF16, tag="t2")
                nc.gpsimd.tensor_mul(t2[:], u_sb[:], _bc(e, [0, (0, O), 1, 2]))
                return t2

            def stage_b(ci, t2):
                # s += sum_r w*u  (in-place tree over r)
                nc.vector.tensor_add(t2[:, :, :8], t2[:, :, :8], t2[:, :, 8:])
                nc.vector.tensor_add(t2[:, :, :4], t2[:, :, :4], t2[:, :, 4:8])
                nc.vector.tensor_add(t2[:, :, :2], t2[:, :, :2], t2[:, :, 2:4])
                nc.vector.tensor_add(t2[:, :, 0], t2[:, :, 0], t2[:, :, 1])
                if ci == 0:
                    nc.vector.tensor_copy(s_acc[:], t2[:, :, 0])
                else:
                    nc.vector.tensor_add(s_acc[:], s_acc[:], t2[:, :, 0])

            f_prev = m_prev = None
            for ci in range(NCT + 2):
                f_cur = stage_f(ci) if ci < NCT else None
                m_cur = stage_m(f_prev) if f_prev is not None else None
                if m_prev is not None:
                    stage_b(ci - 2, m_prev)
                f_prev, m_prev = f_cur, m_cur
            s_stage = acc.tile([B, C * O], F32, tag="s_stage")
            if it == 1:
                allreduce(1, s_acc[:].rearrange("b o c -> b c o"),
                          stage=s_stage)
                v1 = sq.tile([B, C, O], F32, tag="v1")
                _squash_emit(nc, sq, sfull[:], v1[:], bias_eps)
                nc.vector.tensor_add(V[:], V[:], v1[:])
            else:
                nc.vector.tensor_copy(
                    s_stage[:].rearrange("b (c o) -> b c o", o=O),
                    s_acc[:].rearrange("b o c -> b c o"))
                nc.gpsimd.dma_start(out=out_d[:], in_=s_stage[:])
    nc.compile()
    return nc


_PREP_CACHE = {}


def _prep_shards(x, w):
    """x -> [core, 4(m), I, RQ, B] bf16 (r block-grouped: m = r//RQ);
    w -> per-core natural [C, RL, I, O] bf16. Runs on jax-CPU (multithreaded).
    """
    import jax
    import jax.numpy as jnp
    if "fn" not in _PREP_CACHE:
        cpu = jax.devices("cpu")[0]

        def _prep(x, w):
            xr = x.reshape(B, NCORES, 4, RQ, I).transpose(1, 2, 4, 3, 0)
            return xr.astype(jnp.bfloat16), w.astype(jnp.bfloat16)

        _PREP_CACHE["fn"] = jax.jit(_prep, device=cpu)
    xq, w16 = _PREP_CACHE["fn"](x, w)
    xq = np.asarray(xq)
    w16 = np.asarray(w16)
    maps = []
    for core in range(NCORES):
        r0 = core * RL
        maps.append({"xq": xq[core].reshape(32, RQ * B),
                     "wn": np.ascontiguousarray(
                         w16[:, r0:r0 + RL]).reshape(C, RL * I * O)})
    return maps


_NC_CACHE = {}


def _postprocess(results):
    """results: list of per-core output dicts -> full [B, C, O] output."""
    s2 = np.zeros((B, C * O), dtype=np.float32)
    for i in range(NCORES):
        s2 += np.asarray(results[i]["s2"], dtype=np.float32)
    s2 = s2.reshape(B, C, O)
    sq = np.sum(s2 * s2, axis=-1, keepdims=True)
    v = (sq / (1.0 + sq)) * s2 / np.sqrt(sq + 1e-8)
    return v.astype(np.float32)


def kernel(x, route_weights, _trace=False):
    x = np.asarray(x, dtype=np.float32)
    w = np.asarray(route_weights, dtype=np.float32)
    in_maps = _prep_shards(x, w)
    if "nc" not in _NC_CACHE:
        _NC_CACHE["nc"] = build_nc()
    nc = _NC_CACHE["nc"]
    kw = {}
    if _trace:
        kw = dict(trace=True)
    try:
        res = run_bass_kernel_spmd(nc, in_maps, core_ids=list(range(NCORES)), **kw)
    except ModuleNotFoundError:
        res = run_bass_kernel_spmd(nc, in_maps, core_ids=list(range(NCORES)))
    if _trace:
        kernel._last_exec_ns = getattr(res, "exec_time_ns", None)
    return _postprocess(res.results)

